# revision 4
# baseline (speedup 1.0000x reference)
"""Trainium2 Bass kernel for nn_CA_Module (DANet CAM + SE gate).

Reference math (per batch item b):
    q = x[b].reshape(C, N)                         # C=512, N=4096
    energy = q @ q.T                               # [C, C]
    att = softmax(max_row(energy) - energy)        # == softmax(-energy) rows
    out = att @ q                                  # [C, N]
    pooled = concat([mean_n x, mean_n out])        # [2C]
    hidden = relu(pooled @ w1.T + b1)              # [CR]
    se = sigmoid(hidden @ w2.T + b2)               # [C]
    y = se * x + (1 - se) * out

Sharding: data-parallel over B=16 across 8 cores (2 batch items/core).

Device implementation notes:
  - att row c: exp(min_row(energy)[c] - energy[c,:]) / S[c]; the 1/S and
    (1-se) fold into one per-partition scalar g = (1-se)/S applied to the
    raw second matmul output.
  - big matmuls run as float32r (TF32-like, full rate); fp32r inputs must be
    produced by a compute op (rounding producer), so qT tiles come out of the
    PE-transpose PSUM->SBUF copy as f32r, and phase-D rhs tiles are ACT
    Copy-casts of the fp32 q tiles.
  - mean_n out is NOT reduced from the big out tensor: sum_n out_raw = E~ @
    (sum_n q), a [512,512]@[512,1] matmul.
  - sigmoid is computed as exp(-z) -> +1 -> reciprocal to keep the ACT engine
    on the exp table set the whole kernel (table switches cost ~2.7us).
  - w1t is pre-scaled by 1/N on the host so pooled sums need no extra scale.
"""
import os
import threading
import numpy as np

import concourse.bass as bass
import concourse.tile as tile
from concourse import bacc, mybir, masks
from concourse.bass_utils import run_bass_kernel_spmd

B, C, H, W = 16, 512, 64, 64
N = H * W                 # 4096
NCORES = 8
BP = B // NCORES          # batch items per core
CR = C // 8               # 64
P = 128                   # partitions
CB = C // P               # 4 c-blocks
NK = N // P               # 32 n-blocks of 128
NB = N // 512             # 8 n-chunks of 512

f32 = mybir.dt.float32
f32r = mybir.dt.float32r
FT = mybir.ActivationFunctionType
ALU = mybir.AluOpType
AX = mybir.AxisListType

_lock = threading.Lock()
_cached = {}


def _build():
    nc = bacc.Bacc("TRN2", target_bir_lowering=False, debug=False,
                   num_devices=NCORES)

    x_d = nc.dram_tensor("x", [BP, C, N], f32, kind="ExternalInput").ap()
    w1t_d = nc.dram_tensor("w1t", [2 * C, CR], f32, kind="ExternalInput").ap()
    b1_d = nc.dram_tensor("b1", [CR, 1], f32, kind="ExternalInput").ap()
    w2t_d = nc.dram_tensor("w2t", [CR, C], f32, kind="ExternalInput").ap()
    b2n_d = nc.dram_tensor("b2n", [C, 1], f32, kind="ExternalInput").ap()
    ident_d = nc.dram_tensor("ident", [P, P], f32, kind="ExternalInput").ap()
    y_d = nc.dram_tensor("y", [BP, C, N], f32, kind="ExternalOutput").ap()

    with tile.TileContext(nc) as tc:
        _emit(nc, tc, x_d, w1t_d, b1_d, w2t_d, b2n_d, ident_d, y_d)
    nc.compile()
    return nc


def _emit(nc, tc, x_d, w1t_d, b1_d, w2t_d, b2n_d, ident_d, y_d):
    from contextlib import ExitStack
    ctx = ExitStack()
    with ctx:
        consts = ctx.enter_context(tc.tile_pool(name="consts", bufs=1))
        pq = ctx.enter_context(tc.tile_pool(name="pq", bufs=7))
        pqT = ctx.enter_context(tc.tile_pool(name="pqT", bufs=3))
        pE = ctx.enter_context(tc.tile_pool(name="pE", bufs=4))
        pET = ctx.enter_context(tc.tile_pool(name="pET", bufs=8))
        pqr = ctx.enter_context(tc.tile_pool(name="pqr", bufs=8))
        pbl = ctx.enter_context(tc.tile_pool(name="pbl", bufs=3))
        psm = ctx.enter_context(tc.tile_pool(name="psm", bufs=8))
        # PSUM: eps(4) + tps(2) + ops(2) = 8 banks
        peps = ctx.enter_context(
            tc.tile_pool(name="peps", bufs=4, space=bass.MemorySpace.PSUM))
        ptps = ctx.enter_context(
            tc.tile_pool(name="ptps", bufs=2, space=bass.MemorySpace.PSUM))
        pops = ctx.enter_context(
            tc.tile_pool(name="pops", bufs=2, space=bass.MemorySpace.PSUM))

        # ---- constants / weights ----
        ident = consts.tile([P, P], f32, tag="ident")
        nc.sync.dma_start(ident[:], ident_d[:])
        identr = consts.tile([P, P], f32r, tag="identr")
        nc.vector.tensor_copy(identr[:], ident[:])

        w1t_sb = consts.tile([P, 2 * C // P, CR], f32, tag="w1t")
        nc.sync.dma_start(w1t_sb[:], w1t_d.rearrange("(kb p) j -> p kb j", p=P))
        w1tr = consts.tile([P, 2 * C // P, CR], f32r, tag="w1tr")
        nc.vector.tensor_copy(w1tr[:], w1t_sb[:])

        w2t_sb = consts.tile([CR, C], f32, tag="w2t")
        nc.sync.dma_start(w2t_sb[:], w2t_d[:])
        w2tr = consts.tile([CR, C], f32r, tag="w2tr")
        nc.vector.tensor_copy(w2tr[:], w2t_sb[:])

        b1_sb = consts.tile([CR, 1], f32, tag="b1")
        nc.sync.dma_start(b1_sb[:], b1_d[:])
        b2n_sb = consts.tile([P, CB], f32, tag="b2n")
        nc.sync.dma_start(b2n_sb[:], b2n_d.rearrange("(cb p) one -> p (cb one)", p=P))

        for b in range(BP):
            # ---- phase A: load q, pooled-x sums ----
            q = []
            for cb in range(CB):
                qt = pq.tile([P, N], f32, tag="q")
                nc.sync.dma_start(qt[:], x_d[b, cb * P:(cb + 1) * P, :])
                q.append(qt)
            # [P, 2] with col1 = col0 — fp32r matmuls need even free counts
            px = []
            for cb in range(CB):
                pxt = psm.tile([P, 2], f32r, tag="px")
                with nc.allow_low_precision(reason="pooled sums feed SE gate only"):
                    nc.vector.tensor_reduce(pxt[:, 0:1], q[cb][:], axis=AX.X,
                                            op=ALU.add)
                    nc.vector.tensor_copy(pxt[:, 1:2], pxt[:, 0:1])
                px.append(pxt)

            # ---- phase B: energy = qT.T @ qT accumulated over k ----
            E_ps = [peps.tile([P, C], f32, tag="eps", name=f"E_ps_{b}_{i}")
                    for i in range(CB)]
            for k in range(NK):
                t_ps = ptps.tile([P, C], f32, tag="tps")
                for cb in range(CB):
                    nc.tensor.transpose(
                        t_ps[:, cb * P:(cb + 1) * P],
                        q[cb][:, k * P:(k + 1) * P], ident[:])
                qT = pqT.tile([P, C], f32r, tag="qT")
                if k % 2 == 0:
                    nc.scalar.activation(qT[:], t_ps[:], FT.Copy)
                else:
                    nc.vector.tensor_copy(qT[:], t_ps[:])
                for mc in range(CB):
                    nc.tensor.matmul(
                        E_ps[mc][:], qT[:, mc * P:(mc + 1) * P], qT[:],
                        start=(k == 0), stop=(k == NK - 1))

            # ---- phase C: softmax pieces ----
            E_sb, rS = [], []
            for mc in range(CB):
                m_sb = psm.tile([P, 1], f32, tag="m")
                nc.vector.tensor_reduce(m_sb[:], E_ps[mc][:], axis=AX.X, op=ALU.min)
                Et = pE.tile([P, C], f32r, tag="E")
                S_sb = psm.tile([P, 1], f32, tag="S")
                nc.scalar.activation(Et[:], E_ps[mc][:], FT.Exp,
                                     bias=m_sb[:], scale=-1.0, accum_out=S_sb[:])
                rSt = psm.tile([P, 1], f32, tag="rS")
                nc.vector.reciprocal(rSt[:], S_sb[:])
                E_sb.append(Et)
                rS.append(rSt)

            # ---- phase C2: ET = transpose(E) ----
            ET = []
            for db in range(CB):
                et_ps = ptps.tile([P, C], f32r, tag="tps")
                for cb in range(CB):
                    nc.tensor.transpose(
                        et_ps[:, cb * P:(cb + 1) * P],
                        E_sb[cb][:, db * P:(db + 1) * P], identr[:])
                ETt = pET.tile([P, C], f32r, tag="ET")
                if db % 2 == 0:
                    nc.scalar.activation(ETt[:], et_ps[:], FT.Copy)
                else:
                    nc.vector.tensor_copy(ETt[:], et_ps[:])
                ET.append(ETt)

            # ---- phase C3: SE gate ----
            pout = []
            for cb in range(CB):
                pp = pops.tile([P, 2], f32, tag="ops")
                for db in range(CB):
                    nc.tensor.matmul(pp[:], ET[db][:, cb * P:(cb + 1) * P],
                                     px[db][:], start=(db == 0), stop=(db == CB - 1))
                pot = psm.tile([P, 2], f32r, tag="pout")
                with nc.allow_low_precision(reason="SE gate pooled term"):
                    nc.vector.tensor_scalar(out=pot[:], in0=pp[:],
                                            scalar1=rS[cb][:], scalar2=None,
                                            op0=ALU.mult)
                pout.append(pot)

            h_ps = pops.tile([CR, 2], f32, tag="ops")
            rhs_blocks = px + pout
            for kb in range(2 * C // P):
                nc.tensor.matmul(h_ps[:], w1tr[:, kb, :], rhs_blocks[kb][:],
                                 start=(kb == 0), stop=(kb == 2 * C // P - 1))
            h_sb = psm.tile([CR, 2], f32r, tag="h")
            with nc.allow_low_precision(reason="SE hidden"):
                nc.scalar.activation(h_sb[:], h_ps[:], FT.Relu,
                                     bias=b1_sb[:], scale=1.0)

            se, g = [], []
            for cb in range(CB):
                z_ps = pops.tile([P, 2], f32, tag="ops")
                nc.tensor.matmul(z_ps[:], w2tr[:, cb * P:(cb + 1) * P], h_sb[:],
                                 start=True, stop=True)
                # sigmoid(z + b2) = 1 / (1 + exp(-z - b2)); b2n = -b2
                en = psm.tile([P, 1], f32, tag="en")
                nc.scalar.activation(en[:], z_ps[:, 0:1], FT.Exp,
                                     bias=b2n_sb[:, cb:cb + 1], scale=-1.0)
                den = psm.tile([P, 1], f32, tag="den")
                nc.vector.tensor_scalar_add(den[:], en[:], 1.0)
                set_ = psm.tile([P, 1], f32, tag="se")
                nc.vector.reciprocal(set_[:], den[:])
                # g = (1 - se) / S
                onems = psm.tile([P, 1], f32, tag="onems")
                nc.vector.tensor_scalar(out=onems[:], in0=set_[:], scalar1=-1.0,
                                        scalar2=1.0, op0=ALU.mult, op1=ALU.add)
                gt = psm.tile([P, 1], f32, tag="g")
                nc.vector.tensor_mul(gt[:], onems[:], rS[cb][:])
                se.append(set_)
                g.append(gt)

            # ---- phase D: out_raw = ET.T @ q_r; blend ----
            for nb in range(NB):
                qr = []
                for db in range(CB):
                    qrt = pqr.tile([P, 512], f32r, tag="qr")
                    nc.scalar.activation(qrt[:], q[db][:, nb * 512:(nb + 1) * 512],
                                         FT.Copy)
                    qr.append(qrt)
                for cb in range(CB):
                    o_ps = pops.tile([P, 512], f32, tag="ops")
                    for db in range(CB):
                        nc.tensor.matmul(o_ps[:], ET[db][:, cb * P:(cb + 1) * P],
                                         qr[db][:], start=(db == 0),
                                         stop=(db == CB - 1))
                    v = pbl.tile([P, 512], f32, tag="v")
                    nc.scalar.activation(v[:], o_ps[:], FT.Copy, scale=g[cb][:])
                    w_ = pbl.tile([P, 512], f32, tag="w")
                    nc.vector.tensor_scalar_mul(
                        w_[:], q[cb][:, nb * 512:(nb + 1) * 512], se[cb][:])
                    f_ = pbl.tile([P, 512], f32, tag="f")
                    nc.vector.tensor_add(f_[:], v[:], w_[:])
                    nc.sync.dma_start(
                        y_d[b, cb * P:(cb + 1) * P, nb * 512:(nb + 1) * 512], f_[:])


def _get_program():
    with _lock:
        if "nc" not in _cached:
            _cached["nc"] = _build()
    return _cached["nc"]


def _prep_in_maps(x, w1, b1, w2, b2):
    x = np.ascontiguousarray(np.asarray(x, dtype=np.float32)).reshape(B, C, N)
    w1 = np.asarray(w1, dtype=np.float32)
    b1 = np.asarray(b1, dtype=np.float32)
    w2 = np.asarray(w2, dtype=np.float32)
    b2 = np.asarray(b2, dtype=np.float32)

    w1t = np.ascontiguousarray(w1.T) / np.float32(N)      # [1024, 64]
    w2t = np.ascontiguousarray(w2.T)                      # [64, 512]
    b1c = np.ascontiguousarray(b1.reshape(CR, 1))
    b2n = np.ascontiguousarray(-b2.reshape(C, 1))
    ident = np.eye(P, dtype=np.float32)

    in_maps = []
    for c in range(NCORES):
        in_maps.append({
            "x": np.ascontiguousarray(x[c * BP:(c + 1) * BP]),
            "w1t": w1t.astype(np.float32),
            "b1": b1c,
            "w2t": w2t,
            "b2n": b2n,
            "ident": ident,
        })
    return in_maps


def run(x, w1, b1, w2, b2, trace=False):
    nc = _get_program()
    in_maps = _prep_in_maps(x, w1, b1, w2, b2)
    res = run_bass_kernel_spmd(nc, in_maps, core_ids=list(range(NCORES)),
                               trace=trace)
    y = np.concatenate([res.results[c]["y"][None] for c in range(NCORES)], axis=0)
    y = y.reshape(B, C, H, W).astype(np.float32)
    return y, res


def kernel(x, w1, b1, w2, b2):
    y, _ = run(x, w1, b1, w2, b2, trace=False)
    return y
